# revision 4
# baseline (speedup 1.0000x reference)
"""Grouped-query attention (B=1, S=2048, HID=4096, 32 q-heads / 8 kv-heads,
D=128, RoPE, additive causal mask) on 8 Trainium2 NeuronCores.

Sharding: tensor-parallel over heads. Core c owns 4 q-heads (columns
512c:512c+512 of Wq), kv-head c (columns 128c:128c+128 of Wk/Wv), and rows
512c:512c+512 of Wo. Each core emits a full-shape partial of the output
projection; the host sums the 8 partials (the "all-reduce" of the row-sharded
Wo matmul).

On-device layout (per core):
  - x^T [4096, 2048] streamed from DRAM (host passes the transpose).
  - Q^T[h] = Wq_h.T @ x.T and K^T = Wk.T @ x.T computed feature-major so that
    RoPE and the attention matmuls need no further transposes.
  - scores are computed TRANSPOSED: scT[sk, sq] = K_t . Q, so softmax's exp
    output lands directly in P^T layout for the PV matmul. Row sums are taken
    with a ones-vector matmul on the tensor engine (logits are O(5), so no max
    subtraction is needed; fully-masked entries underflow exp() to 0 exactly,
    matching the reference's exp(-1e9 - max) == 0).
  - out^T[d, sq] = sum_t V_t.T @ P^T_t, normalized by 1/l while copying to
    SBUF, is exactly the lhsT the output projection needs.

All matmuls run in float32r (fp32 rounded to 11-bit mantissa, full PE rate at
N=512). Matmul operands are pre-rounded on the host so the HW matmul is exact
on the rounded values.
"""
import numpy as np
from contextlib import ExitStack

import concourse.bass as bass
import concourse.tile as tile
from concourse import bacc, mybir
from concourse.bass_utils import run_bass_kernel_spmd
from concourse.masks import make_identity

F32 = mybir.dt.float32
F32R = mybir.dt.float32r
EXP = mybir.ActivationFunctionType.Exp

S = 2048
HID = 4096
D = 128
NCORES = 8
NHQ = 4                      # q heads per core
SCALE = float(D) ** -0.5
ST = S // 128                # 16 s-tiles
SL = S // 512                # 4 s-slices
KT = HID // 128              # 32 hidden k-tiles
NO = HID // 512              # 8 output column slices

_NC_CACHE = {}


def round_fp32r(a: np.ndarray) -> np.ndarray:
    """Round fp32 to fp32r (1s+8e+11m, top 20 bits), round-to-nearest-even."""
    u = np.ascontiguousarray(a, dtype=np.float32).view(np.uint32)
    low = u & np.uint32(0xFFF)
    base = u & np.uint32(0xFFFFF000)
    lsb = (u >> np.uint32(12)) & np.uint32(1)
    up = (low > np.uint32(0x800)) | ((low == np.uint32(0x800)) & (lsb == np.uint32(1)))
    return (base + np.where(up, np.uint32(0x1000), np.uint32(0))).view(np.float32)


def _rope_drain(nc, dst_slice, src_ps, cs, sn, rot):
    """dst = src*cos + rotate_half(src)*sin, reading PSUM, writing f32r SBUF.

    sn is the sign-folded sin table slice (rows 0:64 pre-negated), so
    rotate_half is just a half-swap read of the PSUM tile.
    """
    nc.vector.tensor_mul(dst_slice, src_ps[:], cs)
    nc.vector.tensor_mul(rot[0:64, :], src_ps[64:128, :], sn[0:64, :])
    nc.vector.tensor_mul(rot[64:128, :], src_ps[0:64, :], sn[64:128, :])
    nc.vector.tensor_add(dst_slice, dst_slice, rot[:])


def build_nc():
    nc = bacc.Bacc("TRN2", target_bir_lowering=False, debug=False,
                   num_devices=NCORES)
    xT = nc.dram_tensor("xT", [HID, S], F32R, kind="ExternalInput").ap()
    wq = nc.dram_tensor("wq", [HID, 128 * NHQ], F32R, kind="ExternalInput").ap()
    wk = nc.dram_tensor("wk", [HID, D], F32R, kind="ExternalInput").ap()
    wv = nc.dram_tensor("wv", [HID, D], F32R, kind="ExternalInput").ap()
    wo = nc.dram_tensor("wo", [128 * NHQ, HID], F32R, kind="ExternalInput").ap()
    maskT = nc.dram_tensor("maskT", [ST, 128, 512], F32, kind="ExternalInput").ap()
    cosT = nc.dram_tensor("cosT", [128, S], F32, kind="ExternalInput").ap()
    sinTf = nc.dram_tensor("sinTf", [128, S], F32, kind="ExternalInput").ap()
    ones = nc.dram_tensor("ones", [128, 1], F32R, kind="ExternalInput").ap()
    y = nc.dram_tensor("y", [S, HID], F32, kind="ExternalOutput").ap()
    # DRAM bounce for the per-(h,j) 1/l row: SBUF sources cannot broadcast
    # across partitions, DRAM sources can (step-0 partition dim).
    rscr = nc.dram_tensor("rscr", [NHQ * SL, 512], F32)

    with tile.TileContext(nc) as tc, ExitStack() as ctx:
        const = ctx.enter_context(tc.tile_pool(name="const", bufs=1))
        cos_sb = const.tile([128, S], F32)
        sin_sb = const.tile([128, S], F32)
        ones_sb = const.tile([128, 1], F32R)
        ident = const.tile([128, 128], F32)
        nc.sync.dma_start(out=cos_sb[:], in_=cosT[:])
        nc.sync.dma_start(out=sin_sb[:], in_=sinTf[:])
        nc.sync.dma_start(out=ones_sb[:], in_=ones[:])
        make_identity(nc, ident[:])

        qt = [const.tile([128, S], F32R, tag=f"qt{h}", name=f"qt{h}") for h in range(NHQ)]
        kt = const.tile([128, S], F32R)
        v_sb = const.tile([128, S], F32R)

        # ---------------- Phase A: projections + RoPE -----------------------
        with tc.tile_pool(name="wA", bufs=1) as wA, \
             tc.tile_pool(name="xtp", bufs=2) as xtp, \
             tc.tile_pool(name="psA", bufs=1, space="PSUM") as psA, \
             tc.tile_pool(name="psAv", bufs=2, space="PSUM") as psAv, \
             tc.tile_pool(name="ropes", bufs=2) as ropes, \
             tc.tile_pool(name="vtmp", bufs=2) as vtmp:
            wq_sb = wA.tile([128, KT, 128 * NHQ], F32R)
            wk_sb = wA.tile([128, KT, D], F32R)
            wv_sb = wA.tile([128, KT, D], F32R)
            nc.sync.dma_start(out=wq_sb[:], in_=wq.rearrange("(t p) f -> p t f", p=128))
            nc.sync.dma_start(out=wk_sb[:], in_=wk.rearrange("(t p) f -> p t f", p=128))
            nc.sync.dma_start(out=wv_sb[:], in_=wv.rearrange("(t p) f -> p t f", p=128))

            for j in range(SL):
                qps = [psA.tile([128, 512], F32, tag=f"qps{f}", name=f"qps{f}") for f in range(NHQ)]
                kps = psA.tile([128, 512], F32, tag="kps")
                vps = psA.tile([128, 512], F32, tag="vps")
                for g in range(KT // 4):
                    xt = xtp.tile([128, 4, 512], F32R, tag="xt")
                    nc.sync.dma_start(
                        out=xt[:],
                        in_=xT[512 * g:512 * (g + 1), 512 * j:512 * (j + 1)]
                        .rearrange("(t p) m -> p t m", p=128),
                    )
                    for kk in range(4):
                        k = 4 * g + kk
                        rhs = xt[:, kk, :]
                        st, sp = (k == 0), (k == KT - 1)
                        for f in range(NHQ):
                            nc.tensor.matmul(
                                qps[f][:], wq_sb[:, k, 128 * f:128 * (f + 1)],
                                rhs, start=st, stop=sp)
                        nc.tensor.matmul(kps[:], wk_sb[:, k, :], rhs, start=st, stop=sp)
                        nc.tensor.matmul(vps[:], wv_sb[:, k, :], rhs, start=st, stop=sp)

                cs = cos_sb[:, 512 * j:512 * (j + 1)]
                sn = sin_sb[:, 512 * j:512 * (j + 1)]
                for h in range(NHQ):
                    rot = ropes.tile([128, 512], F32, tag="rot")
                    _rope_drain(nc, qt[h][:, 512 * j:512 * (j + 1)], qps[h], cs, sn, rot)
                rot = ropes.tile([128, 512], F32, tag="rot")
                _rope_drain(nc, kt[:, 512 * j:512 * (j + 1)], kps, cs, sn, rot)

                # V^T psum -> V [s, d] sbuf via PE transpose
                vts = vtmp.tile([128, 512], F32, tag="vts")
                nc.scalar.copy(vts[:], vps[:])
                for t2 in range(4):
                    vtp = psAv.tile([128, 128], F32, tag="vtp")
                    nc.tensor.transpose(vtp[:], vts[:, 128 * t2:128 * (t2 + 1)], ident[:])
                    nc.scalar.copy(
                        v_sb[:, 128 * (4 * j + t2):128 * (4 * j + t2 + 1)], vtp[:])

        # outT lives from phase B through phase C; allocated after the phase-A
        # weight pools are freed so SBUF peak stays under budget.
        persist = ctx.enter_context(tc.tile_pool(name="persist", bufs=1))
        outT = [persist.tile([128, S], F32R, tag=f"outT{h}", name=f"outT{h}")
                for h in range(NHQ)]

        # ---------------- Phase B: attention --------------------------------
        with tc.tile_pool(name="maskp", bufs=1) as maskp, \
             tc.tile_pool(name="ptbp", bufs=6) as ptbp, \
             tc.tile_pool(name="psB", bufs=4, space="PSUM") as psB, \
             tc.tile_pool(name="psL", bufs=2, space="PSUM") as psL, \
             tc.tile_pool(name="psO", bufs=2, space="PSUM") as psO, \
             tc.tile_pool(name="rbcp", bufs=2) as rbcp:
            mask_sb = maskp.tile([128, ST, 512], F32)
            nc.sync.dma_start(out=mask_sb[:], in_=maskT.rearrange("t p q -> p t q"))

            for h in range(NHQ):
                for j in range(SL):
                    tmax = 4 * j + 4
                    lps = psL.tile([1, 512], F32, tag="lps")
                    ops = psO.tile([128, 512], F32, tag="ops")
                    pts = [None] * tmax
                    # software-pipelined by one: sc/exp(t) then lsum/pv(t-1)
                    for t in range(tmax + 1):
                        if t < tmax:
                            scp = psB.tile([128, 512], F32, tag="scp")
                            nc.tensor.matmul(
                                scp[:], kt[:, 128 * t:128 * (t + 1)],
                                qt[h][:, 512 * j:512 * (j + 1)],
                                start=True, stop=True, skip_group_check=True)
                            if t >= 4 * j:  # diagonal block: additive mask
                                nc.vector.tensor_add(scp[:], scp[:], mask_sb[:, t, :])
                            ptb = ptbp.tile([128, 512], F32R, tag="ptb")
                            nc.scalar.activation(ptb[:], scp[:], EXP,
                                                 bias=0.0, scale=SCALE)
                            pts[t] = ptb
                        if t >= 1:
                            u = t - 1
                            nc.tensor.matmul(
                                lps[:], ones_sb[:], pts[u][:],
                                start=(u == 0), stop=(u == tmax - 1),
                                skip_group_check=True)
                            nc.tensor.matmul(
                                ops[:], v_sb[:, 128 * u:128 * (u + 1)], pts[u][:],
                                start=(u == 0), stop=(u == tmax - 1),
                                skip_group_check=True)
                    rsb = rbcp.tile([1, 512], F32, tag="rsb")
                    nc.vector.reciprocal(rsb[:], lps[:])
                    hj = h * SL + j
                    nc.sync.dma_start(out=rscr[hj:hj + 1, :], in_=rsb[:])
                    rb = rbcp.tile([128, 512], F32, tag="rb")
                    nc.sync.dma_start(
                        out=rb[:], in_=rscr[hj:hj + 1, :].partition_broadcast(128))
                    nc.vector.tensor_mul(
                        outT[h][:, 512 * j:512 * (j + 1)], ops[:], rb[:])

        # ---------------- Phase C: output projection ------------------------
        with tc.tile_pool(name="wop", bufs=3) as wop, \
             tc.tile_pool(name="ysbp", bufs=4) as ysbp, \
             tc.tile_pool(name="psC", bufs=4, space="PSUM") as psC:
            for n in range(NO):
                wo_t = wop.tile([128, NHQ, 512], F32R, tag="wo")
                nc.sync.dma_start(
                    out=wo_t[:],
                    in_=wo[:, 512 * n:512 * (n + 1)]
                    .rearrange("(k p) o -> p k o", p=128))
                for sq in range(ST):
                    yp = psC.tile([128, 512], F32, tag="yp")
                    for k in range(NHQ):
                        nc.tensor.matmul(
                            yp[:], outT[k][:, 128 * sq:128 * (sq + 1)],
                            wo_t[:, k, :], start=(k == 0), stop=(k == NHQ - 1))
                    ys = ysbp.tile([128, 512], F32, tag="ys")
                    if sq % 2 == 0:
                        nc.vector.tensor_copy(ys[:], yp[:])
                    else:
                        nc.scalar.copy(ys[:], yp[:])
                    nc.sync.dma_start(
                        out=y[128 * sq:128 * (sq + 1), 512 * n:512 * (n + 1)],
                        in_=ys[:])

    nc.compile()
    return nc


def get_nc():
    if "nc" not in _NC_CACHE:
        _NC_CACHE["nc"] = build_nc()
    return _NC_CACHE["nc"]


def make_in_maps(hidden_states, attention_mask, position_ids, Wq, Wk, Wv, Wo):
    hs = np.asarray(hidden_states, dtype=np.float32)
    mask = np.asarray(attention_mask, dtype=np.float32)
    pos = np.asarray(position_ids)
    Wq = np.asarray(Wq, dtype=np.float32)
    Wk = np.asarray(Wk, dtype=np.float32)
    Wv = np.asarray(Wv, dtype=np.float32)
    Wo = np.asarray(Wo, dtype=np.float32)
    assert hs.shape == (1, S, HID) and mask.shape == (1, 1, S, S)
    assert Wq.shape == (HID, HID) and Wk.shape == (HID, 1024)
    assert Wv.shape == (HID, 1024) and Wo.shape == (HID, HID)

    xT = round_fp32r(hs[0].T)
    m2 = mask[0, 0]
    maskT = np.stack([
        np.ascontiguousarray(m2[512 * (t // 4):512 * (t // 4 + 1),
                                128 * t:128 * (t + 1)].T) / np.float32(SCALE)
        for t in range(ST)
    ]).astype(np.float32)

    p = pos[0].astype(np.float32)
    inv = (1.0 / (10000.0 ** (np.arange(0, D, 2, dtype=np.float32)
                              / np.float32(D)))).astype(np.float32)
    freqs = p[:, None] * inv[None, :]
    emb = np.concatenate([freqs, freqs], axis=1)        # (S, 128)
    cosT = np.ascontiguousarray(np.cos(emb).T).astype(np.float32)
    sinT = np.sin(emb).T.astype(np.float32)
    sinTf = sinT.copy()
    sinTf[:64] *= np.float32(-1.0)
    sinTf = np.ascontiguousarray(sinTf)
    ones = np.ones((128, 1), dtype=np.float32)

    in_maps = []
    for c in range(NCORES):
        in_maps.append({
            "xT": xT,
            "wq": round_fp32r(Wq[:, 512 * c:512 * (c + 1)]),
            "wk": round_fp32r(Wk[:, 128 * c:128 * (c + 1)]),
            "wv": round_fp32r(Wv[:, 128 * c:128 * (c + 1)]),
            "wo": round_fp32r(Wo[512 * c:512 * (c + 1), :]),
            "maskT": maskT,
            "cosT": cosT,
            "sinTf": sinTf,
            "ones": ones,
        })
    return in_maps


def kernel(hidden_states, attention_mask, position_ids, Wq, Wk, Wv, Wo):
    in_maps = make_in_maps(hidden_states, attention_mask, position_ids,
                           Wq, Wk, Wv, Wo)
    nc = get_nc()
    res = run_bass_kernel_spmd(nc, in_maps, list(range(NCORES)))
    acc = np.zeros((S, HID), dtype=np.float64)
    for c in range(NCORES):
        acc += res.results[c]["y"]
    return acc.astype(np.float32)[None]
